# revision 63
# baseline (speedup 1.0000x reference)
"""
AllegroConditioner Trainium2 kernel (8-core data parallel), v2.4.

Algorithmic core: every edge's neighbor-sum contribution is a fixed 64-dim
function g(d) of the scalar edge distance (g(d>=5) == 0 smoothly).  We expand
g over the quarter-wave cosine family
    psi_j(d) = cos((2j-1) * pi * d / 10),   j = 1..J        (J = 17)
the eigenbasis for {even in d, zero at d=5}: g is even in d and vanishes
cubically at the cutoff, so the expansion converges ~ k^-4.  The
(basis -> latent -> densenet) mixing D @ w2 @ wd0 is folded into the first
densenet layer on the host, so the moments
    P_j[s,i] = sum_p U[p,i] * psi_j[p,s]
enter the densenet directly.  The host fit runs against an op-for-op
emulation of the device chain, absorbing systematic fp16 drift.

Clip trick: r = relu(5 - d) on ACT; psi_1 = sin(pi/10 r), c2 = sin(pi/5 r -
pi/2) = cos(pi d/5) are in-range ACT sins, and all psi_j are exactly 0 at
r=0, so the cutoff mask is free.  Chebyshev chain psi_j = C psi_{j-1} -
psi_{j-2} (C = 2 c2) runs full-tile fp16 on the DVE (2x perf mode).

Layout tricks:
  - x lands pre-rearranged [128, 4*256] via one DMA; two PE transposes per
    sample chunk: cols 0:128 (xrest head) and cols 128:256 (xrest tail +
    interleaved cartesian rows 3a+k).  The pair-difference matrix is built
    on the 96-row interleaved layout directly, so no coordinate unpacking.
  - weights land k-tile-packed from the host (one DMA each).
  - a tiny Silu decoy activation pins the silu_and_others ACT table set
    (sin + tanh + copy + square + relu) so the whole back half of the kernel
    runs without table reloads (hardware build only; CoreSim lacks Silu).
  - L2/L3 accumulate per k-tile as tanh outputs arrive.
"""

import math
import numpy as np
import ml_dtypes

import concourse.bass as bass
import concourse.bacc as bacc
import concourse.mybir as mybir
import concourse.tile as tile
from concourse import masks
from concourse.bass_utils import run_bass_kernel_spmd

# ---------------- problem constants ----------------
N_CORES = 8
B_FULL = 4096
BC = B_FULL // N_CORES          # 512 samples per core
DIM_IN = 256
N_ATOMS = 32
REST = DIM_IN - 3 * N_ATOMS     # 160
CUT = 5.0
LAT = 64
HID = 512
DOUT = 256
NB = 8

NPAIR = (N_ATOMS * (N_ATOMS - 1)) // 2   # 496 unordered pairs
PBLK = 4                                  # pair blocks of 128 (512 slots, 16 pad)
SCHUNK = 4                                # sample chunks of 128

J_MODES = 16                              # quarter-wave cosine count
KT_L1 = 6                                 # L1 k-tiles (see wl1 layout below)
NCART = 3 * N_ATOMS                       # 96 interleaved cartesian rows

F32 = mybir.dt.float32
BF16 = mybir.dt.bfloat16
F16 = mybir.dt.float16

_PAIR_I, _PAIR_J = np.triu_indices(N_ATOMS, 1)


# ---------------- host-side fit ----------------

def _emulate_basis(d):
    """Op-for-op emulation of the device basis chain as a function of d."""
    f16 = np.float16
    d64 = np.asarray(d, np.float32).astype(np.float64)
    r = np.maximum(5.0 - d64, 0.0)                      # ACT relu (f32)
    psi1 = np.asarray(np.sin(np.pi / 10 * r), f16).astype(np.float64)
    c2 = np.asarray(np.sin(np.pi / 5 * r - np.pi / 2), f16).astype(np.float64)
    C = np.asarray(2.0 * c2, f16).astype(np.float64)
    Cm1 = np.asarray(2.0 * c2 - 1.0, f16).astype(np.float64)
    psi = [psi1, np.asarray(Cm1 * psi1, f16).astype(np.float64)]
    for j in range(3, J_MODES + 1):
        u = np.asarray(C * psi[-1], f16).astype(np.float64)
        psi.append(np.asarray(u - psi[-2], f16).astype(np.float64))
    return np.stack(psi, -1)                            # [N, J]


def _fit_basis(w1, b1, x):
    """Fit g(d)=silu(feat@w1+b1) onto the emulated device basis, [J, LAT]."""
    gr = np.linspace(0.001, CUT, 6000)
    evr = np.where(gr < CUT,
                   1 - 10 * (gr / CUT) ** 3 + 15 * (gr / CUT) ** 4 - 6 * (gr / CUT) ** 5,
                   0.0) / np.maximum(gr, 1e-9)
    n = np.arange(1, NB + 1)
    feat = np.sin(n * np.pi * gr[:, None] / CUT) * evr[:, None]
    t = feat @ w1.astype(np.float64) + b1.astype(np.float64)
    g = t / (1.0 + np.exp(-t))

    xs = np.asarray(x[:256], np.float64)
    xc = xs[:, REST:].reshape(-1, N_ATOMS, 3)
    dd = np.sqrt(((xc[:, _PAIR_I] - xc[:, _PAIR_J]) ** 2).sum(-1)).ravel()
    dd = dd[dd < CUT]
    hist, edges = np.histogram(dd, bins=50, range=(0, CUT), density=True)
    w = np.interp(gr, 0.5 * (edges[1:] + edges[:-1]), hist) + 0.05
    sw = np.sqrt(w)[:, None]

    Phi = _emulate_basis(gr)
    lam = 3e-3
    A = np.vstack([Phi * sw, lam * np.eye(J_MODES)])
    Y = np.vstack([g * sw, np.zeros((J_MODES, LAT))])
    D, *_ = np.linalg.lstsq(A, Y, rcond=None)
    return D                                            # [J, LAT]


def _pack_host(inputs):
    """Host-side weight folding. Returns dict of device arrays (shared by cores)."""
    w1 = np.asarray(inputs["w1"], np.float64)
    b1 = np.asarray(inputs["b1"], np.float64)
    w2 = np.asarray(inputs["w2"], np.float64)
    wd0 = np.asarray(inputs["wd0"], np.float64)
    D = _fit_basis(w1, b1, np.asarray(inputs["x"], np.float32))
    CW = D @ w2                                         # [J, LAT]

    # L1 stationary, 6 k-tiles of 128 rows:
    #   kt0 = xrest[0:128]
    #   kt1 = modes 1..3 (rows 0:96) + xrest[128:160] (rows 96:128)
    #   kt2..kt4 = modes 4..15;  kt5 = modes 16..J + zero pad
    wl1 = np.zeros((KT_L1 * 128, HID), np.float64)
    wl1[0:128, :] = wd0[0:128, :]
    wl1[224:256, :] = wd0[128:REST, :]

    def mode_row(m):
        if m <= 3:
            return 128 + 32 * (m - 1)
        return 256 + 32 * (m - 4)

    for m in range(1, J_MODES + 1):
        base = mode_row(m)
        for i in range(N_ATOMS):
            wl1[base + i, :] = CW[m - 1] @ wd0[REST + LAT * i: REST + LAT * (i + 1), :]

    # pair difference matrix on the interleaved 96-row layout:
    #   row 3a+k <-> cartesian coordinate (atom a, axis k);  per-axis column
    #   blocks of 512 pair slots.  Plus the pair->atom scatter U^T.
    dmat96 = np.zeros((NCART, 3 * PBLK * 128), np.float32)
    umat = np.zeros((128, PBLK * 32), np.float32)
    for p in range(NPAIR):
        t, pl = divmod(p, 128)
        i, j = _PAIR_I[p], _PAIR_J[p]
        for k in range(3):
            dmat96[3 * i + k, 512 * k + 128 * t + pl] = 1.0
            dmat96[3 * j + k, 512 * k + 128 * t + pl] = -1.0
        umat[pl, 32 * t + i] = 1.0
        umat[pl, 32 * t + j] = 1.0

    def ktile_pack(a, nkt, width):
        return np.ascontiguousarray(
            a.reshape(nkt, 128, width).transpose(1, 0, 2).reshape(128, nkt * width))

    bf = ml_dtypes.bfloat16
    f16 = np.float16
    cblob = np.zeros((128, 3 * PBLK * 128 + PBLK * 32), f16)   # dmat96 | umat
    cblob[:NCART, :3 * PBLK * 128] = dmat96.astype(f16)
    cblob[:, 3 * PBLK * 128:] = umat.astype(f16)
    fblob = np.zeros((128, 4 + 4 + DOUT), np.float32)          # bd0 | bd1 | bd2
    fblob[:, 0:4] = np.asarray(inputs["bd0"], np.float32).reshape(4, 128).T
    fblob[:, 4:8] = np.asarray(inputs["bd1"], np.float32).reshape(4, 128).T
    fblob[:, 8:] = np.broadcast_to(np.asarray(inputs["bd2"], np.float32), (128, DOUT))
    return {
        "wl1": ktile_pack(wl1.astype(f16), KT_L1, HID),
        "wd1": ktile_pack(np.asarray(inputs["wd1"], np.float32).astype(bf), 4, HID),
        "wd2": ktile_pack(np.asarray(inputs["wd2"], np.float32).astype(bf), 4, DOUT),
        "cblob": np.ascontiguousarray(cblob),
        "fblob": np.ascontiguousarray(fblob),
    }


# ---------------- device kernel ----------------

def build_nc(hw=True):
    nc = bacc.Bacc(target_bir_lowering=False, debug=False)

    x_ext = nc.declare_dram_parameter("x", [128, SCHUNK * DIM_IN], F32, isOutput=False)
    wl1_ext = nc.declare_dram_parameter("wl1", [128, KT_L1 * HID], F16, isOutput=False)
    wd1_ext = nc.declare_dram_parameter("wd1", [128, 4 * HID], BF16, isOutput=False)
    wd2_ext = nc.declare_dram_parameter("wd2", [128, 4 * DOUT], BF16, isOutput=False)
    cblob_ext = nc.declare_dram_parameter("cblob", [128, 12 * 128 + PBLK * 32], F16,
                                          isOutput=False)
    fblob_ext = nc.declare_dram_parameter("fblob", [128, 8 + DOUT], F32, isOutput=False)
    out_ext = nc.declare_dram_parameter("out", [BC, DIM_IN], F32, isOutput=True)

    AF = mybir.ActivationFunctionType
    ALU = mybir.AluOpType

    with tile.TileContext(nc) as tc:
        with (
            tc.tile_pool(name="const", bufs=1) as constp,
            tc.tile_pool(name="persist", bufs=1) as persist,
            tc.tile_pool(name="schain", bufs=8) as schain,
            tc.tile_pool(name="uchain", bufs=3) as uchain,
            tc.tile_pool(name="work", bufs=3) as work,
        ):
            # ---- consts ----
            eps_sb = constp.tile([128, 1], F32)
            nc.vector.memset(eps_sb[:], 1e-12)
            hpi_sb = constp.tile([128, 1], F32)
            nc.vector.memset(hpi_sb[:], math.pi / 2)
            identf = constp.tile([128, 128], F32)
            masks.make_identity(nc, identf[:])
            identb = constp.tile([128, 128], BF16)
            masks.make_identity(nc, identb[:])
            scratch = constp.tile([128, 1], F32)

            # sqrt decoy: pins sqrt_and_others (copy/square/relu fillers) for
            # the whole front half of the kernel
            nc.scalar.activation(scratch[:], eps_sb[:], AF.Sqrt)

            # ---- load x FIRST (4 chunk DMAs on two queues) ----
            xt = persist.tile([128, SCHUNK * DIM_IN], F32, tag="xt")
            for c in range(SCHUNK):
                eng = nc.sync if c % 2 == 0 else nc.scalar
                eng.dma_start(xt[:, DIM_IN * c:DIM_IN * (c + 1)],
                              x_ext[:, DIM_IN * c:DIM_IN * (c + 1)])

            cblob_sb = constp.tile([128, 12 * 128 + PBLK * 32], F16)
            nc.gpsimd.dma_start(cblob_sb[:], cblob_ext[:])
            dmat_sb = cblob_sb[:NCART, :12 * 128]
            umat_sb = cblob_sb[:, 12 * 128:]
            fblob_sb = constp.tile([128, 8 + DOUT], F32)
            nc.gpsimd.dma_start(fblob_sb[:], fblob_ext[:])
            bd0_sb = fblob_sb[:, 0:4]
            bd1_sb = fblob_sb[:, 4:8]
            bd2_sb = fblob_sb[:, 8:]
            wl1_sb = constp.tile([128, KT_L1 * HID], F16)
            wd1_sb = constp.tile([128, 4 * HID], BF16)
            wd2_sb = constp.tile([128, 4 * DOUT], BF16)

            # ---- L1 rhs k-tiles (f16) ----
            kt_tiles = [persist.tile([128, BC], F16, tag=f"kt{k}", name=f"kt{k}")
                        for k in range(KT_L1)]
            lastrows = 32 * ((J_MODES - 4) % 4 + 1)
            if lastrows < 64:
                nc.vector.memset(kt_tiles[5][lastrows:64, :], 0.0)
                nc.vector.memset(kt_tiles[5][64:128, :], 0.0)
            else:
                nc.vector.memset(kt_tiles[5][lastrows:128, :], 0.0)

            # ---- transposes into two batched psum tiles ----
            xc_hi = persist.tile([NCART, BC], F16, tag="xch")
            xc_lo = persist.tile([NCART, BC], F16, tag="xcl")
            rt = persist.tile([128, PBLK * BC], F32, tag="rt")
            with (
                tc.tile_pool(name="ps_tr", bufs=1, space="PSUM") as ps_tr,
                tc.tile_pool(name="ps_warm", bufs=1, space="PSUM") as ps_warm,
            ):
                wsrc = constp.tile([128, 512], BF16)
                nc.vector.memset(wsrc[:], 0.5)
                for _ in range(3):
                    pw = ps_warm.tile([128, 512], F32, tag="w")
                    nc.tensor.matmul(pw[:], identb[:], wsrc[:],
                                     start=True, stop=True)
                tA = ps_tr.tile([128, BC], F32, tag="tA", name="tA")
                tB = ps_tr.tile([128, BC], F32, tag="tB", name="tB")
                for c in range(SCHUNK):
                    xc_ = xt[:, DIM_IN * c:DIM_IN * (c + 1)]
                    nc.tensor.matmul(tA[:, 128 * c:128 * (c + 1)], xc_[:, 0:128],
                                     identf[:], is_transpose=True,
                                     skip_group_check=True)
                    nc.tensor.matmul(tB[:, 128 * c:128 * (c + 1)], xc_[:, 128:256],
                                     identf[:], is_transpose=True,
                                     skip_group_check=True)
                # drains: kt0, kt1 tail rows, cartesian hi/lo split
                # (x cols are host-permuted per chunk: [rest0 128 | cart 96 | rest1 32],
                #  so tB rows 0:96 = cartesian, 96:128 = xrest tail)
                nc.scalar.copy(kt_tiles[0][:], tA[:])
                nc.scalar.copy(kt_tiles[1][96:128, :], tB[96:128, :])
                nc.scalar.copy(xc_hi[:], tB[0:96, :])
                nc.vector.tensor_tensor(xc_lo[:], tB[0:96, :], xc_hi[:],
                                        ALU.subtract)

            # ---- distances -> clipped d, [128, 2048] f32 ----
            # weight DMAs issue here: they transfer during the distance phase
            # without starving the x/cblob loads above
            nc.gpsimd.dma_start(wl1_sb[:], wl1_ext[:])
            nc.gpsimd.dma_start(wd1_sb[:], wd1_ext[:])
            nc.gpsimd.dma_start(wd2_sb[:], wd2_ext[:])
            with tc.tile_pool(name="ps_diff", bufs=2, space="PSUM") as ps_diff:
                for t in range(PBLK):
                    psd = ps_diff.tile([128, 3 * BC], F32, tag="diff")
                    for k in range(3):
                        nc.tensor.matmul(
                            psd[:, BC * k:BC * (k + 1)],
                            dmat_sb[:, 512 * k + 128 * t:512 * k + 128 * (t + 1)],
                            xc_hi[:], start=True, stop=False, skip_group_check=True)
                        nc.tensor.matmul(
                            psd[:, BC * k:BC * (k + 1)],
                            dmat_sb[:, 512 * k + 128 * t:512 * k + 128 * (t + 1)],
                            xc_lo[:], start=False, stop=True, skip_group_check=True)
                    sq = work.tile([128, 3 * BC], F32, tag="sq")
                    nc.scalar.square(sq[:], psd[:])
                    d2s = work.tile([128, BC], F32, tag="d2s")
                    nc.vector.tensor_tensor(d2s[:], sq[:, 0:BC], sq[:, BC:2 * BC],
                                            ALU.add)
                    nc.vector.tensor_tensor(d2s[:], d2s[:], sq[:, 2 * BC:3 * BC],
                                            ALU.add)
                    nc.vector.tensor_scalar_min(d2s[:], d2s[:], 25.0)
                    nc.scalar.activation(rt[:, BC * t:BC * (t + 1)], d2s[:],
                                         AF.Sqrt, bias=eps_sb[:])

            # ---- chain seeds directly from clipped d ----
            c2t = persist.tile([128, PBLK * BC], F16, tag="c2")
            nc.scalar.activation(c2t[:], rt[:], AF.Sin, scale=-math.pi / 5,
                                 bias=hpi_sb[:])
            psi1 = schain.tile([128, PBLK * BC], F16, tag="psi")
            nc.scalar.activation(psi1[:], rt[:], AF.Sin, scale=-math.pi / 10,
                                 bias=hpi_sb[:])
            # tanh decoy anchored on psi1 (so the scheduler cannot hoist it):
            # the exp_and_others load lands here, hidden under the chain
            # phase, and z1/z2 need no reload
            if hw:
                nc.scalar.activation(scratch[:], psi1[:, 0:1], AF.Tanh)
            Cm1t = persist.tile([128, PBLK * BC], F16, tag="Cm1")
            Ct = persist.tile([128, PBLK * BC], F16, tag="C")
            nc.vector.tensor_scalar(Cm1t[:], c2t[:], 2.0, -1.0, ALU.mult, ALU.add)
            nc.vector.tensor_scalar_mul(Ct[:], c2t[:], 2.0)

            # ---- chain + moments + L1 ----
            z1 = persist.tile([128, 4 * BC], BF16, tag="z1")
            z2 = persist.tile([128, 4 * BC], BF16, tag="z2")
            with tc.tile_pool(name="ps_l1", bufs=1, space="PSUM") as ps_l1:
                ps1_tiles = [ps_l1.tile([128, BC], F32, tag=f"l1_{mt}",
                                        name=f"l1_{mt}") for mt in range(4)]

                def l1_accum(kt, start, stop):
                    for mt in range(4):
                        nc.tensor.matmul(
                            ps1_tiles[mt][:],
                            wl1_sb[:, HID * kt + 128 * mt: HID * kt + 128 * (mt + 1)],
                            kt_tiles[kt][:],
                            start=start, stop=stop)

                with tc.tile_pool(name="ps_mom", bufs=2, space="PSUM") as ps_mom:
                    l1_accum(0, True, False)

                    def group_slot(m):
                        if m <= 3:
                            return 0, m - 1
                        return (m - 4) // 4 + 1, (m - 4) % 4

                    s_tiles = {1: psi1}
                    psi2 = schain.tile([128, PBLK * BC], F16, tag="psi")
                    nc.vector.tensor_tensor(psi2[:], Cm1t[:], psi1[:], ALU.mult)
                    s_tiles[2] = psi2

                    mom_tiles = {}
                    for m in range(1, J_MODES + 1):
                        g, ms = group_slot(m)
                        if m >= 3:
                            u = uchain.tile([128, PBLK * BC], F16, tag="u")
                            nc.vector.tensor_tensor(u[:], Ct[:], s_tiles[m - 1][:],
                                                    ALU.mult)
                            sm = schain.tile([128, PBLK * BC], F16, tag="psi")
                            nc.vector.tensor_tensor(sm[:], u[:], s_tiles[m - 2][:],
                                                    ALU.subtract)
                            s_tiles[m] = sm
                        if ms == 0:
                            mom_tiles[g] = ps_mom.tile([128, BC], F32, tag="mom",
                                                       name=f"mom{g}")
                        psm = mom_tiles[g]
                        ph = s_tiles[m]
                        for t in range(PBLK):
                            nc.tensor.matmul(
                                psm[32 * ms:32 * (ms + 1), :],
                                umat_sb[:, 32 * t:32 * (t + 1)],
                                ph[:, BC * t:BC * (t + 1)],
                                start=(t == 0), stop=(t == PBLK - 1),
                                tile_position=(0, 32 * ms),
                                skip_group_check=True)
                        if m == 3:
                            nc.scalar.copy(kt_tiles[1][:96, :], mom_tiles[0][:96, :])
                            l1_accum(1, False, False)
                        elif m >= 4 and ms == 3:
                            nc.scalar.copy(kt_tiles[1 + g][:], mom_tiles[g][:])
                            l1_accum(1 + g, False, False)
                        elif m == J_MODES:
                            nc.scalar.copy(kt_tiles[5][:lastrows, :],
                                           mom_tiles[g][:lastrows, :])
                            l1_accum(5, False, True)

                # ---- z1 + L2, accumulated per arriving k-tile ----
                with tc.tile_pool(name="ps_fin2", bufs=1, space="PSUM") as ps_fin2:
                    ps2_tiles = [ps_fin2.tile([128, BC], F32, tag=f"fin{mt}",
                                              name=f"fin{mt}") for mt in range(4)]
                    for kt in range(4):
                        nc.scalar.activation(z1[:, BC * kt:BC * (kt + 1)],
                                             ps1_tiles[kt][:], AF.Tanh,
                                             bias=bd0_sb[:, kt:kt + 1])
                        for mt in range(4):
                            nc.tensor.matmul(
                                ps2_tiles[mt][:],
                                wd1_sb[:, HID * kt + 128 * mt: HID * kt + 128 * (mt + 1)],
                                z1[:, BC * kt:BC * (kt + 1)],
                                start=(kt == 0), stop=(kt == 3))
                    # z2 tanh happens inside this pool; L3 runs after it closes
                    for kt in range(4):
                        nc.scalar.activation(z2[:, BC * kt:BC * (kt + 1)],
                                             ps2_tiles[kt][:], AF.Tanh,
                                             bias=bd1_sb[:, kt:kt + 1])

            # ---- L3 + output ----
            with tc.tile_pool(name="ps_fin3", bufs=2, space="PSUM") as ps_fin3:
                for c in range(SCHUNK):
                    ps3 = ps_fin3.tile([128, DOUT], F32, tag="o")
                    for kt in range(4):
                        nc.tensor.matmul(
                            ps3[:],
                            z2[:, BC * kt + 128 * c: BC * kt + 128 * (c + 1)],
                            wd2_sb[:, DOUT * kt:DOUT * (kt + 1)],
                            start=(kt == 0), stop=(kt == 3))
                    ot = work.tile([128, DOUT], F32, tag="ot")
                    nc.vector.tensor_tensor(ot[:], ps3[:], bd2_sb[:], ALU.add)
                    nc.sync.dma_start(out_ext[128 * c:128 * (c + 1), :], ot[:])

    nc.compile()
    return nc


_CACHE = {}


def kernel(**inputs) -> np.ndarray:
    x = np.ascontiguousarray(np.asarray(inputs["x"], np.float32))
    packed = _pack_host(inputs)
    if "nc" not in _CACHE:
        _CACHE["nc"] = build_nc(hw=True)
    nc = _CACHE["nc"]
    in_maps = []
    perm = np.concatenate([np.arange(0, 128), np.arange(REST, DIM_IN),
                           np.arange(128, REST)])
    for c in range(N_CORES):
        m = dict(packed)
        xc = x[BC * c:BC * (c + 1), :][:, perm]
        m["x"] = np.ascontiguousarray(
            xc.reshape(SCHUNK, 128, DIM_IN).transpose(1, 0, 2).reshape(128, -1))
        in_maps.append(m)
    res = run_bass_kernel_spmd(nc, in_maps, core_ids=list(range(N_CORES)))
    _CACHE["last_exec_ns"] = getattr(res, "exec_time_ns", None)
    outs = [res.results[c]["out"] for c in range(N_CORES)]
    return np.concatenate(outs, axis=0).astype(np.float32)


if __name__ == "__main__":
    rng = np.random.default_rng(0)
    fake = {
        "x": rng.standard_normal((B_FULL, DIM_IN)).astype(np.float32),
        "w1": (rng.standard_normal((NB, LAT)) / np.sqrt(NB)).astype(np.float32),
        "b1": np.zeros(LAT, np.float32),
        "w2": (rng.standard_normal((LAT, LAT)) / np.sqrt(LAT)).astype(np.float32),
        "b2": np.zeros(LAT, np.float32),
        "wd0": (rng.standard_normal((REST + N_ATOMS * LAT, HID)) / 47.0).astype(np.float32),
        "bd0": np.zeros(HID, np.float32),
        "wd1": (rng.standard_normal((HID, HID)) / np.sqrt(HID)).astype(np.float32),
        "bd1": np.zeros(HID, np.float32),
        "wd2": (rng.standard_normal((HID, DOUT)) / np.sqrt(HID)).astype(np.float32),
        "bd2": np.zeros(DOUT, np.float32),
    }
    fake["x"][:, REST:] *= 3.0
    out = kernel(**fake)
    print("kernel out:", out.shape, out.dtype, np.abs(out).mean())


# revision 65
# speedup vs baseline: 1.2323x; 1.2323x over previous
"""
AllegroConditioner Trainium2 kernel (8-core data parallel), final.

Algorithmic core: every edge's neighbor-sum contribution is a fixed 64-dim
function g(d) of the scalar edge distance (g(d>=5) == 0 smoothly).  We expand
g over the quarter-wave cosine family
    psi_j(d) = cos((2j-1) * pi * d / 10),   j = 1..J        (J = 16)
the eigenbasis for {even in d, zero at d=5}: g is even in d and vanishes
cubically at the cutoff, so the expansion converges ~ k^-4.  The
(basis -> latent -> densenet) mixing D @ w2 @ wd0 is folded into the first
densenet layer on the host, so the moments
    P_j[s,i] = sum_p U[p,i] * psi_j[p,s]
enter the densenet directly.  The host fit runs against an op-for-op
emulation of the device chain, absorbing systematic fp16 drift.

Clip trick: d2 is clamped to 25 on the DVE, so d = sqrt(d2) <= 5 and the
seeds psi_1 = sin(pi/2 - pi/10 d), c2 = sin(pi/2 - pi/5 d) = cos(pi d/5)
are in-range ACT sins with all psi_j exactly 0 at d=5 — the cutoff mask is
free.  Chebyshev chain psi_j = C psi_{j-1} - psi_{j-2} (C = 2 c2) runs
full-tile fp16 on the DVE (2x perf mode).

Layout tricks:
  - x lands host-permuted per chunk ([rest0 128 | cart 96 | rest1 32]); two
    PE transposes per sample chunk into two batched PSUM tiles; the
    pair-difference matrix is built on the interleaved 96-row cartesian
    layout directly, so no coordinate unpacking.
  - weights land k-tile-packed from the host (one DMA each), issued after
    the x/cblob loads so they never starve them.
  - ACT table sets: a sqrt decoy pinned first covers copy/square for the
    whole front; a tanh decoy anchored on psi1 pulls the exp_and_others
    load under the chain phase, so z1/z2 need no reload.  Only the
    trig_and_small load (before the seeds) stays on the critical path.
  - L2 accumulates per arriving z1 k-tile; moments/L1 interleave with the
    chain; keep-warm matmuls cover the initial DMA wait (they are
    load-bearing: trimming them costs ~20us in downstream PE clock ramp).
"""

import math
import numpy as np
import ml_dtypes

import concourse.bass as bass
import concourse.bacc as bacc
import concourse.mybir as mybir
import concourse.tile as tile
from concourse import masks
from concourse.bass_utils import run_bass_kernel_spmd

# ---------------- problem constants ----------------
N_CORES = 8
B_FULL = 4096
BC = B_FULL // N_CORES          # 512 samples per core
DIM_IN = 256
N_ATOMS = 32
REST = DIM_IN - 3 * N_ATOMS     # 160
CUT = 5.0
LAT = 64
HID = 512
DOUT = 256
NB = 8

NPAIR = (N_ATOMS * (N_ATOMS - 1)) // 2   # 496 unordered pairs
PBLK = 4                                  # pair blocks of 128 (512 slots, 16 pad)
SCHUNK = 4                                # sample chunks of 128

J_MODES = 16                              # quarter-wave cosine count
KT_L1 = 6                                 # L1 k-tiles (see wl1 layout below)
NCART = 3 * N_ATOMS                       # 96 interleaved cartesian rows

F32 = mybir.dt.float32
BF16 = mybir.dt.bfloat16
F16 = mybir.dt.float16

_PAIR_I, _PAIR_J = np.triu_indices(N_ATOMS, 1)


# ---------------- host-side fit ----------------

def _emulate_basis(d):
    """Op-for-op emulation of the device basis chain as a function of d."""
    f16 = np.float16
    d64 = np.asarray(d, np.float32).astype(np.float64)
    r = np.maximum(5.0 - d64, 0.0)                      # ACT relu (f32)
    psi1 = np.asarray(np.sin(np.pi / 10 * r), f16).astype(np.float64)
    c2 = np.asarray(np.sin(np.pi / 5 * r - np.pi / 2), f16).astype(np.float64)
    C = np.asarray(2.0 * c2, f16).astype(np.float64)
    Cm1 = np.asarray(2.0 * c2 - 1.0, f16).astype(np.float64)
    psi = [psi1, np.asarray(Cm1 * psi1, f16).astype(np.float64)]
    for j in range(3, J_MODES + 1):
        u = np.asarray(C * psi[-1], f16).astype(np.float64)
        psi.append(np.asarray(u - psi[-2], f16).astype(np.float64))
    return np.stack(psi, -1)                            # [N, J]


def _fit_basis(w1, b1, x):
    """Fit g(d)=silu(feat@w1+b1) onto the emulated device basis, [J, LAT]."""
    gr = np.linspace(0.001, CUT, 6000)
    evr = np.where(gr < CUT,
                   1 - 10 * (gr / CUT) ** 3 + 15 * (gr / CUT) ** 4 - 6 * (gr / CUT) ** 5,
                   0.0) / np.maximum(gr, 1e-9)
    n = np.arange(1, NB + 1)
    feat = np.sin(n * np.pi * gr[:, None] / CUT) * evr[:, None]
    t = feat @ w1.astype(np.float64) + b1.astype(np.float64)
    g = t / (1.0 + np.exp(-t))

    xs = np.asarray(x[:256], np.float64)
    xc = xs[:, REST:].reshape(-1, N_ATOMS, 3)
    dd = np.sqrt(((xc[:, _PAIR_I] - xc[:, _PAIR_J]) ** 2).sum(-1)).ravel()
    dd = dd[dd < CUT]
    hist, edges = np.histogram(dd, bins=50, range=(0, CUT), density=True)
    w = np.interp(gr, 0.5 * (edges[1:] + edges[:-1]), hist) + 0.05
    sw = np.sqrt(w)[:, None]

    Phi = _emulate_basis(gr)
    lam = 3e-3
    A = np.vstack([Phi * sw, lam * np.eye(J_MODES)])
    Y = np.vstack([g * sw, np.zeros((J_MODES, LAT))])
    D, *_ = np.linalg.lstsq(A, Y, rcond=None)
    return D                                            # [J, LAT]


def _pack_host(inputs):
    """Host-side weight folding. Returns dict of device arrays (shared by cores)."""
    w1 = np.asarray(inputs["w1"], np.float64)
    b1 = np.asarray(inputs["b1"], np.float64)
    w2 = np.asarray(inputs["w2"], np.float64)
    wd0 = np.asarray(inputs["wd0"], np.float64)
    D = _fit_basis(w1, b1, np.asarray(inputs["x"], np.float32))
    CW = D @ w2                                         # [J, LAT]

    # L1 stationary, 6 k-tiles of 128 rows:
    #   kt0 = xrest[0:128]
    #   kt1 = modes 1..3 (rows 0:96) + xrest[128:160] (rows 96:128)
    #   kt2..kt4 = modes 4..15;  kt5 = modes 16..J + zero pad
    wl1 = np.zeros((KT_L1 * 128, HID), np.float64)
    wl1[0:128, :] = wd0[0:128, :]
    wl1[224:256, :] = wd0[128:REST, :]

    def mode_row(m):
        if m <= 3:
            return 128 + 32 * (m - 1)
        return 256 + 32 * (m - 4)

    for m in range(1, J_MODES + 1):
        base = mode_row(m)
        for i in range(N_ATOMS):
            wl1[base + i, :] = CW[m - 1] @ wd0[REST + LAT * i: REST + LAT * (i + 1), :]

    # pair difference matrix on the interleaved 96-row layout:
    #   row 3a+k <-> cartesian coordinate (atom a, axis k);  per-axis column
    #   blocks of 512 pair slots.  Plus the pair->atom scatter U^T.
    dmat96 = np.zeros((NCART, 3 * PBLK * 128), np.float32)
    umat = np.zeros((128, PBLK * 32), np.float32)
    for p in range(NPAIR):
        t, pl = divmod(p, 128)
        i, j = _PAIR_I[p], _PAIR_J[p]
        for k in range(3):
            dmat96[3 * i + k, 512 * k + 128 * t + pl] = 1.0
            dmat96[3 * j + k, 512 * k + 128 * t + pl] = -1.0
        umat[pl, 32 * t + i] = 1.0
        umat[pl, 32 * t + j] = 1.0

    def ktile_pack(a, nkt, width):
        return np.ascontiguousarray(
            a.reshape(nkt, 128, width).transpose(1, 0, 2).reshape(128, nkt * width))

    bf = ml_dtypes.bfloat16
    f16 = np.float16
    cblob = np.zeros((128, 3 * PBLK * 128 + PBLK * 32), f16)   # dmat96 | umat
    cblob[:NCART, :3 * PBLK * 128] = dmat96.astype(f16)
    cblob[:, 3 * PBLK * 128:] = umat.astype(f16)
    fblob = np.zeros((128, 4 + 4 + DOUT), np.float32)          # bd0 | bd1 | bd2
    fblob[:, 0:4] = np.asarray(inputs["bd0"], np.float32).reshape(4, 128).T
    fblob[:, 4:8] = np.asarray(inputs["bd1"], np.float32).reshape(4, 128).T
    fblob[:, 8:] = np.broadcast_to(np.asarray(inputs["bd2"], np.float32), (128, DOUT))
    return {
        "wl1": ktile_pack(wl1.astype(f16), KT_L1, HID),
        "wd1": ktile_pack(np.asarray(inputs["wd1"], np.float32).astype(bf), 4, HID),
        "wd2": ktile_pack(np.asarray(inputs["wd2"], np.float32).astype(bf), 4, DOUT),
        "cblob": np.ascontiguousarray(cblob),
        "fblob": np.ascontiguousarray(fblob),
    }


# ---------------- device kernel ----------------

def build_nc(hw=True):
    nc = bacc.Bacc(target_bir_lowering=False, debug=False)

    x_ext = nc.declare_dram_parameter("x", [128, SCHUNK * DIM_IN], F32, isOutput=False)
    wl1_ext = nc.declare_dram_parameter("wl1", [128, KT_L1 * HID], F16, isOutput=False)
    wd1_ext = nc.declare_dram_parameter("wd1", [128, 4 * HID], BF16, isOutput=False)
    wd2_ext = nc.declare_dram_parameter("wd2", [128, 4 * DOUT], BF16, isOutput=False)
    cblob_ext = nc.declare_dram_parameter("cblob", [128, 12 * 128 + PBLK * 32], F16,
                                          isOutput=False)
    fblob_ext = nc.declare_dram_parameter("fblob", [128, 8 + DOUT], F32, isOutput=False)
    out_ext = nc.declare_dram_parameter("out", [BC, DIM_IN], F32, isOutput=True)

    AF = mybir.ActivationFunctionType
    ALU = mybir.AluOpType

    with tile.TileContext(nc) as tc:
        with (
            tc.tile_pool(name="const", bufs=1) as constp,
            tc.tile_pool(name="persist", bufs=1) as persist,
            tc.tile_pool(name="schain", bufs=8) as schain,
            tc.tile_pool(name="uchain", bufs=3) as uchain,
            tc.tile_pool(name="work", bufs=3) as work,
        ):
            # ---- consts ----
            eps_sb = constp.tile([128, 1], F32)
            nc.vector.memset(eps_sb[:], 1e-12)
            hpi_sb = constp.tile([128, 1], F32)
            nc.vector.memset(hpi_sb[:], math.pi / 2)
            identf = constp.tile([128, 128], F32)
            masks.make_identity(nc, identf[:])
            identb = constp.tile([128, 128], BF16)
            masks.make_identity(nc, identb[:])
            scratch = constp.tile([128, 1], F32)

            # sqrt decoy: pins sqrt_and_others (copy/square/relu fillers) for
            # the whole front half of the kernel
            nc.scalar.activation(scratch[:], eps_sb[:], AF.Sqrt)

            # ---- load x FIRST (4 chunk DMAs on two queues) ----
            xt = persist.tile([128, SCHUNK * DIM_IN], F32, tag="xt")
            for c in range(SCHUNK):
                eng = nc.sync if c % 2 == 0 else nc.scalar
                eng.dma_start(xt[:, DIM_IN * c:DIM_IN * (c + 1)],
                              x_ext[:, DIM_IN * c:DIM_IN * (c + 1)])

            cblob_sb = constp.tile([128, 12 * 128 + PBLK * 32], F16)
            nc.gpsimd.dma_start(cblob_sb[:], cblob_ext[:])
            dmat_sb = cblob_sb[:NCART, :12 * 128]
            umat_sb = cblob_sb[:, 12 * 128:]
            fblob_sb = constp.tile([128, 8 + DOUT], F32)
            nc.gpsimd.dma_start(fblob_sb[:], fblob_ext[:])
            bd0_sb = fblob_sb[:, 0:4]
            bd1_sb = fblob_sb[:, 4:8]
            bd2_sb = fblob_sb[:, 8:]
            wl1_sb = constp.tile([128, KT_L1 * HID], F16)
            wd1_sb = constp.tile([128, 4 * HID], BF16)
            wd2_sb = constp.tile([128, 4 * DOUT], BF16)

            # ---- L1 rhs k-tiles (f16) ----
            kt_tiles = [persist.tile([128, BC], F16, tag=f"kt{k}", name=f"kt{k}")
                        for k in range(KT_L1)]
            lastrows = 32 * ((J_MODES - 4) % 4 + 1)
            if lastrows < 64:
                nc.vector.memset(kt_tiles[5][lastrows:64, :], 0.0)
                nc.vector.memset(kt_tiles[5][64:128, :], 0.0)
            else:
                nc.vector.memset(kt_tiles[5][lastrows:128, :], 0.0)

            # ---- transposes into two batched psum tiles ----
            xc_hi = persist.tile([NCART, BC], F16, tag="xch")
            xc_lo = persist.tile([NCART, BC], F16, tag="xcl")
            rt = persist.tile([128, PBLK * BC], F32, tag="rt")
            with (
                tc.tile_pool(name="ps_tr", bufs=1, space="PSUM") as ps_tr,
                tc.tile_pool(name="ps_warm", bufs=1, space="PSUM") as ps_warm,
            ):
                wsrc = constp.tile([128, 512], BF16)
                nc.vector.memset(wsrc[:], 0.5)
                for _ in range(6):
                    pw = ps_warm.tile([128, 512], F32, tag="w")
                    nc.tensor.matmul(pw[:], identb[:], wsrc[:],
                                     start=True, stop=True)
                tA = ps_tr.tile([128, BC], F32, tag="tA", name="tA")
                tB = ps_tr.tile([128, BC], F32, tag="tB", name="tB")
                for c in range(SCHUNK):
                    xc_ = xt[:, DIM_IN * c:DIM_IN * (c + 1)]
                    nc.tensor.matmul(tA[:, 128 * c:128 * (c + 1)], xc_[:, 0:128],
                                     identf[:], is_transpose=True,
                                     skip_group_check=True)
                    nc.tensor.matmul(tB[:, 128 * c:128 * (c + 1)], xc_[:, 128:256],
                                     identf[:], is_transpose=True,
                                     skip_group_check=True)
                # drains: kt0, kt1 tail rows, cartesian hi/lo split
                # (x cols are host-permuted per chunk: [rest0 128 | cart 96 | rest1 32],
                #  so tB rows 0:96 = cartesian, 96:128 = xrest tail)
                nc.scalar.copy(kt_tiles[0][:], tA[:])
                nc.scalar.copy(kt_tiles[1][96:128, :], tB[96:128, :])
                nc.scalar.copy(xc_hi[:], tB[0:96, :])
                nc.vector.tensor_tensor(xc_lo[:], tB[0:96, :], xc_hi[:],
                                        ALU.subtract)

            # ---- distances -> clipped d, [128, 2048] f32 ----
            # weight DMAs issue here: they transfer during the distance phase
            # without starving the x/cblob loads above
            nc.gpsimd.dma_start(wl1_sb[:], wl1_ext[:])
            nc.gpsimd.dma_start(wd1_sb[:], wd1_ext[:])
            nc.gpsimd.dma_start(wd2_sb[:], wd2_ext[:])
            with tc.tile_pool(name="ps_diff", bufs=2, space="PSUM") as ps_diff:
                for t in range(PBLK):
                    psd = ps_diff.tile([128, 3 * BC], F32, tag="diff")
                    for k in range(3):
                        nc.tensor.matmul(
                            psd[:, BC * k:BC * (k + 1)],
                            dmat_sb[:, 512 * k + 128 * t:512 * k + 128 * (t + 1)],
                            xc_hi[:], start=True, stop=False, skip_group_check=True)
                        nc.tensor.matmul(
                            psd[:, BC * k:BC * (k + 1)],
                            dmat_sb[:, 512 * k + 128 * t:512 * k + 128 * (t + 1)],
                            xc_lo[:], start=False, stop=True, skip_group_check=True)
                    sq = work.tile([128, 3 * BC], F32, tag="sq")
                    nc.scalar.square(sq[:], psd[:])
                    d2s = work.tile([128, BC], F32, tag="d2s")
                    nc.vector.tensor_tensor(d2s[:], sq[:, 0:BC], sq[:, BC:2 * BC],
                                            ALU.add)
                    nc.vector.tensor_tensor(d2s[:], d2s[:], sq[:, 2 * BC:3 * BC],
                                            ALU.add)
                    nc.vector.tensor_scalar_min(d2s[:], d2s[:], 25.0)
                    nc.scalar.activation(rt[:, BC * t:BC * (t + 1)], d2s[:],
                                         AF.Sqrt, bias=eps_sb[:])

            # ---- chain seeds directly from clipped d ----
            c2t = persist.tile([128, PBLK * BC], F16, tag="c2")
            nc.scalar.activation(c2t[:], rt[:], AF.Sin, scale=-math.pi / 5,
                                 bias=hpi_sb[:])
            psi1 = schain.tile([128, PBLK * BC], F16, tag="psi")
            nc.scalar.activation(psi1[:], rt[:], AF.Sin, scale=-math.pi / 10,
                                 bias=hpi_sb[:])
            # tanh decoy anchored on psi1 (so the scheduler cannot hoist it):
            # the exp_and_others load lands here, hidden under the chain
            # phase, and z1/z2 need no reload
            if hw:
                nc.scalar.activation(scratch[:], psi1[:, 0:1], AF.Tanh)
            Cm1t = persist.tile([128, PBLK * BC], F16, tag="Cm1")
            Ct = persist.tile([128, PBLK * BC], F16, tag="C")
            nc.vector.tensor_scalar(Cm1t[:], c2t[:], 2.0, -1.0, ALU.mult, ALU.add)
            nc.vector.tensor_scalar_mul(Ct[:], c2t[:], 2.0)

            # ---- chain + moments + L1 ----
            z1 = persist.tile([128, 4 * BC], BF16, tag="z1")
            z2 = persist.tile([128, 4 * BC], BF16, tag="z2")
            with tc.tile_pool(name="ps_l1", bufs=1, space="PSUM") as ps_l1:
                ps1_tiles = [ps_l1.tile([128, BC], F32, tag=f"l1_{mt}",
                                        name=f"l1_{mt}") for mt in range(4)]

                def l1_accum(kt, start, stop):
                    for mt in range(4):
                        nc.tensor.matmul(
                            ps1_tiles[mt][:],
                            wl1_sb[:, HID * kt + 128 * mt: HID * kt + 128 * (mt + 1)],
                            kt_tiles[kt][:],
                            start=start, stop=stop)

                with tc.tile_pool(name="ps_mom", bufs=2, space="PSUM") as ps_mom:
                    l1_accum(0, True, False)

                    def group_slot(m):
                        if m <= 3:
                            return 0, m - 1
                        return (m - 4) // 4 + 1, (m - 4) % 4

                    s_tiles = {1: psi1}
                    psi2 = schain.tile([128, PBLK * BC], F16, tag="psi")
                    nc.vector.tensor_tensor(psi2[:], Cm1t[:], psi1[:], ALU.mult)
                    s_tiles[2] = psi2

                    mom_tiles = {}
                    for m in range(1, J_MODES + 1):
                        g, ms = group_slot(m)
                        if m >= 3:
                            u = uchain.tile([128, PBLK * BC], F16, tag="u")
                            nc.vector.tensor_tensor(u[:], Ct[:], s_tiles[m - 1][:],
                                                    ALU.mult)
                            sm = schain.tile([128, PBLK * BC], F16, tag="psi")
                            nc.vector.tensor_tensor(sm[:], u[:], s_tiles[m - 2][:],
                                                    ALU.subtract)
                            s_tiles[m] = sm
                        if ms == 0:
                            mom_tiles[g] = ps_mom.tile([128, BC], F32, tag="mom",
                                                       name=f"mom{g}")
                        psm = mom_tiles[g]
                        ph = s_tiles[m]
                        for t in range(PBLK):
                            nc.tensor.matmul(
                                psm[32 * ms:32 * (ms + 1), :],
                                umat_sb[:, 32 * t:32 * (t + 1)],
                                ph[:, BC * t:BC * (t + 1)],
                                start=(t == 0), stop=(t == PBLK - 1),
                                tile_position=(0, 32 * ms),
                                skip_group_check=True)
                        if m == 3:
                            nc.scalar.copy(kt_tiles[1][:96, :], mom_tiles[0][:96, :])
                            l1_accum(1, False, False)
                        elif m >= 4 and ms == 3:
                            nc.scalar.copy(kt_tiles[1 + g][:], mom_tiles[g][:])
                            l1_accum(1 + g, False, False)
                        elif m == J_MODES:
                            nc.scalar.copy(kt_tiles[5][:lastrows, :],
                                           mom_tiles[g][:lastrows, :])
                            l1_accum(5, False, True)

                # ---- z1 + L2, accumulated per arriving k-tile ----
                with tc.tile_pool(name="ps_fin2", bufs=1, space="PSUM") as ps_fin2:
                    ps2_tiles = [ps_fin2.tile([128, BC], F32, tag=f"fin{mt}",
                                              name=f"fin{mt}") for mt in range(4)]
                    for kt in range(4):
                        nc.scalar.activation(z1[:, BC * kt:BC * (kt + 1)],
                                             ps1_tiles[kt][:], AF.Tanh,
                                             bias=bd0_sb[:, kt:kt + 1])
                        for mt in range(4):
                            nc.tensor.matmul(
                                ps2_tiles[mt][:],
                                wd1_sb[:, HID * kt + 128 * mt: HID * kt + 128 * (mt + 1)],
                                z1[:, BC * kt:BC * (kt + 1)],
                                start=(kt == 0), stop=(kt == 3))
                    # z2 tanh happens inside this pool; L3 runs after it closes
                    for kt in range(4):
                        nc.scalar.activation(z2[:, BC * kt:BC * (kt + 1)],
                                             ps2_tiles[kt][:], AF.Tanh,
                                             bias=bd1_sb[:, kt:kt + 1])

            # ---- L3 + output ----
            with tc.tile_pool(name="ps_fin3", bufs=2, space="PSUM") as ps_fin3:
                for c in range(SCHUNK):
                    ps3 = ps_fin3.tile([128, DOUT], F32, tag="o")
                    for kt in range(4):
                        nc.tensor.matmul(
                            ps3[:],
                            z2[:, BC * kt + 128 * c: BC * kt + 128 * (c + 1)],
                            wd2_sb[:, DOUT * kt:DOUT * (kt + 1)],
                            start=(kt == 0), stop=(kt == 3))
                    ot = work.tile([128, DOUT], F32, tag="ot")
                    nc.vector.tensor_tensor(ot[:], ps3[:], bd2_sb[:], ALU.add)
                    nc.sync.dma_start(out_ext[128 * c:128 * (c + 1), :], ot[:])

    nc.compile()
    return nc


_CACHE = {}


def kernel(**inputs) -> np.ndarray:
    x = np.ascontiguousarray(np.asarray(inputs["x"], np.float32))
    packed = _pack_host(inputs)
    if "nc" not in _CACHE:
        _CACHE["nc"] = build_nc(hw=True)
    nc = _CACHE["nc"]
    in_maps = []
    perm = np.concatenate([np.arange(0, 128), np.arange(REST, DIM_IN),
                           np.arange(128, REST)])
    for c in range(N_CORES):
        m = dict(packed)
        xc = x[BC * c:BC * (c + 1), :][:, perm]
        m["x"] = np.ascontiguousarray(
            xc.reshape(SCHUNK, 128, DIM_IN).transpose(1, 0, 2).reshape(128, -1))
        in_maps.append(m)
    res = run_bass_kernel_spmd(nc, in_maps, core_ids=list(range(N_CORES)))
    _CACHE["last_exec_ns"] = getattr(res, "exec_time_ns", None)
    outs = [res.results[c]["out"] for c in range(N_CORES)]
    return np.concatenate(outs, axis=0).astype(np.float32)


if __name__ == "__main__":
    rng = np.random.default_rng(0)
    fake = {
        "x": rng.standard_normal((B_FULL, DIM_IN)).astype(np.float32),
        "w1": (rng.standard_normal((NB, LAT)) / np.sqrt(NB)).astype(np.float32),
        "b1": np.zeros(LAT, np.float32),
        "w2": (rng.standard_normal((LAT, LAT)) / np.sqrt(LAT)).astype(np.float32),
        "b2": np.zeros(LAT, np.float32),
        "wd0": (rng.standard_normal((REST + N_ATOMS * LAT, HID)) / 47.0).astype(np.float32),
        "bd0": np.zeros(HID, np.float32),
        "wd1": (rng.standard_normal((HID, HID)) / np.sqrt(HID)).astype(np.float32),
        "bd1": np.zeros(HID, np.float32),
        "wd2": (rng.standard_normal((HID, DOUT)) / np.sqrt(HID)).astype(np.float32),
        "bd2": np.zeros(DOUT, np.float32),
    }
    fake["x"][:, REST:] *= 3.0
    out = kernel(**fake)
    print("kernel out:", out.shape, out.dtype, np.abs(out).mean())
